# revision 27
# baseline (speedup 1.0000x reference)
"""Multi-head self-attention (B=2, S=2048, D=1024, H=16) on 8 trn2 cores.

Sharding: data-parallel over B (2) x tensor-parallel over head groups (4 groups
of 4 heads).  Core c handles batch c//4, heads (c%4)*4..(c%4)*4+3.  Each core
computes its partial output projection (over its 256 of the 1024 contraction
columns); the host sums the 4 partials per batch and adds the bias terms.

Structure (per core):
  - K/V projections upfront (2-bank PSUM slots, fused evictions).
  - Per 512-query chunk: scores (K=64 matmuls into a [128,2heads,512]
    pair-block), one 1024-wide exp on ScalarE per block, ctx accumulation
    (exp @ [V | ones] gives softmax denominators for free), normalization
    via reciprocal + GpSimd partition-broadcast.
  - The previous chunk's output projection is interleaved into the current
    chunk's kt loop so the in-order PE queue always has independent work
    (keeps the HAM clock gate at 8/8).
"""

import sys

if "/opt/trn_rl_repo" not in sys.path:
    sys.path.insert(0, "/opt/trn_rl_repo")

from contextlib import ExitStack

import ml_dtypes
import numpy as np

import concourse.bass as bass
import concourse.mybir as mybir
import concourse.tile as tile
from concourse import bacc

F32 = mybir.dt.float32
F32R = mybir.dt.float32r
BF16 = mybir.dt.bfloat16
F16 = mybir.dt.float16
EXP = mybir.ActivationFunctionType.Exp

N_CORES = 8
S = 2048          # sequence length
D = 1024          # model dim
GH = 4            # heads per core
DK = 64           # head dim
E = GH * DK       # projection cols per core (256)
DT = D // 128     # contraction tiles over model dim (8)
KT = S // 128     # key chunks (16)
QC = S // 512     # query chunks (4)
DT_ORDER = [4, 5, 6, 7, 0, 1, 2, 3]   # x dt-chunk arrival order (gpsimd first)


def _build():
    nc = bacc.Bacc("TRN2", target_bir_lowering=False, debug=False,
                   enable_asserts=False, num_devices=N_CORES)

    # Inputs are pre-swizzled host-side to partition-major layouts so every
    # DMA reads large contiguous per-partition lines from HBM.
    xT_d = nc.dram_tensor("xT", [128, 2, DT, 1024], F16, kind="ExternalInput").ap()
    wqT_d = nc.dram_tensor("wqT", [128, DT, E], F16, kind="ExternalInput").ap()
    wkT_d = nc.dram_tensor("wkT", [128, DT, E], F16, kind="ExternalInput").ap()
    wvT_d = nc.dram_tensor("wvT", [128, DT, E], F16, kind="ExternalInput").ap()
    woT_d = nc.dram_tensor("woT", [128, 2, D], F16, kind="ExternalInput").ap()
    bq_d = nc.dram_tensor("bq", [E], F32, kind="ExternalInput").ap()
    bk_d = nc.dram_tensor("bk", [E], F32, kind="ExternalInput").ap()
    out_d = nc.dram_tensor("out", [S, D], F16, kind="ExternalOutput").ap()

    with tile.TileContext(nc) as tc, ExitStack() as ctx:
        const = ctx.enter_context(tc.tile_pool(name="const", bufs=1))

        xT_s = const.tile([128, DT, S], F16, name="xT_s")
        wqT_s = const.tile([128, DT, E], F16, name="wqT_s")
        wkT_s = const.tile([128, DT, E], F16, name="wkT_s")
        wvT_s = const.tile([128, DT, E], F16, name="wvT_s")
        woT_s = const.tile([128, 2, D], F16, name="woT_s")
        bq_s = const.tile([128, 2], F32, name="bq_s")
        bk_s = const.tile([128, 2], F32, name="bk_s")
        # Q and ctx live in per-qc tiles to avoid cross-qc WAR serialization
        QT_q = [const.tile([128, 2, 512], F16, name=f"QT{i}") for i in range(QC)]
        ctx_q = [[const.tile([128, 512], F16, name=f"CT{i}_{p}") for p in range(2)]
                 for i in range(QC)]
        KT_p = [const.tile([128, 1024], F16, name=f"KTp{i}") for i in range(4)]
        # V' layout: [k-partition, k-chunk, head-major (64 V cols + ones col)]
        Vp_t = [const.tile([128, 4, GH * 128], F16, name=f"Vp{i}")
                for i in range(4)]

        # Batched DMAs: contiguous >=512KB per-partition-major sources, split
        # across the two DMA queues so x-lo (kproj00/qproj0) lands first,
        # x-hi next (kproj01 at ~u7 of the stream), weights by first use.
        nc.sync.dma_start(wkT_s[:], wkT_d[:])
        nc.sync.dma_start(bk_s[:, :], bk_d.rearrange("(c p) -> p c", p=128))
        nc.sync.dma_start(bq_s[:, :], bq_d.rearrange("(c p) -> p c", p=128))
        nc.sync.dma_start(xT_s[:, 0:2, 0:1024], xT_d[:, 0, 0:2, :])
        nc.sync.dma_start(xT_s[:, 2:4, 0:1024], xT_d[:, 0, 2:4, :])
        nc.sync.dma_start(xT_s[:, 0:2, 1024:2048], xT_d[:, 1, 0:2, :])
        nc.sync.dma_start(xT_s[:, 2:4, 1024:2048], xT_d[:, 1, 2:4, :])
        nc.gpsimd.dma_start(xT_s[:, 4, 0:1024], xT_d[:, 0, 4, :])
        nc.gpsimd.dma_start(xT_s[:, 5, 0:1024], xT_d[:, 0, 5, :])
        nc.gpsimd.dma_start(xT_s[:, 6:8, 0:1024], xT_d[:, 0, 6:8, :])
        nc.gpsimd.dma_start(wqT_s[:], wqT_d[:])
        nc.gpsimd.dma_start(wvT_s[:], wvT_d[:])
        nc.gpsimd.dma_start(xT_s[:, 4:6, 1024:2048], xT_d[:, 1, 4:6, :])
        nc.gpsimd.dma_start(xT_s[:, 6:8, 1024:2048], xT_d[:, 1, 6:8, :])
        nc.gpsimd.dma_start(woT_s[:], woT_d[:])

        # V' per head: [ones, 63 x zero, V(64)] -> denom at psum partition 0,
        # ctx at partitions 64..127 (32-aligned for DVE reads)
        for vt in Vp_t:
            nc.gpsimd.memset(vt[:], 0.0)
            for hh in range(GH):
                nc.vector.memset(vt[:, :, hh * 128], 1.0)

        def kproj_half(ec, kq, half, pool, pname):
            ps = pool.tile([128, 2, 512], F32, name=pname)
            qc = kq * 2 + half
            for i, dt_ in enumerate(DT_ORDER):
                nc.tensor.matmul(
                    ps[:, 0, :],
                    wkT_s[:, dt_, ec * 128:(ec + 1) * 128],
                    xT_s[:, dt_, qc * 512:(qc + 1) * 512],
                    start=(i == 0), stop=(i == DT - 1))
            nc.vector.tensor_scalar_add(
                KT_p[ec * 2 + kq].rearrange("p (a b) -> p a b", a=2)[:, half, :],
                ps[:, 0, :], bk_s[:, ec:ec + 1])

        def kproj_kq(ec, kq, pool, pname):
            kproj_half(ec, kq, 0, pool, pname)
            kproj_half(ec, kq, 1, pool, pname)

        def vproj_half(sg, half, pool, pname):
            ps = pool.tile([128, 2, 512], F32, name=pname)
            sc = sg * 2 + half
            for i, dt_ in enumerate(DT_ORDER):
                nc.tensor.matmul(
                    ps[:, 0, :E],
                    xT_s[:, dt_, sc * 128:(sc + 1) * 128],
                    wvT_s[:, dt_, :],
                    start=(i == 0), stop=(i == DT - 1))
            vt, so = Vp_t[sg // 2], (sg % 2) * 2 + half
            nc.vector.tensor_copy(
                vt[:, so:so + 1, :]
                .rearrange("p s (h d) -> p s h d", d=128)[:, :, :, DK:128],
                ps[:, 0:1, :E].rearrange("p s (h d) -> p s h d", d=DK))

        def vproj_sg(sg, pool, pname):
            vproj_half(sg, 0, pool, pname)
            vproj_half(sg, 1, pool, pname)

        def qproj_ec(qc, ec, pool=None, pname="sblk"):
            ps = (pool or sp).tile([128, 2, 512], F32, name=pname)
            for i, dt_ in enumerate(DT_ORDER):
                nc.tensor.matmul(
                    ps[:, ec, :],
                    wqT_s[:, dt_, ec * 128:(ec + 1) * 128],
                    xT_s[:, dt_, qc * 512:(qc + 1) * 512],
                    start=(i == 0), stop=(i == DT - 1))
            nc.vector.tensor_scalar_add(
                QT_q[qc][:, ec, :], ps[:, ec, :], bq_s[:, ec:ec + 1])

        def qproj(qc, pool=None, pname="sblk"):
            qproj_ec(qc, 0, pool, pname)
            qproj_ec(qc, 1, pool, pname)

        # ---- upfront projections (own 2-slot psum pool, closed afterwards) ----
        with tc.tile_pool(name="kv", bufs=2, space="PSUM") as kv:
            kproj_kq(0, 0, kv, "kvt")
            qproj_ec(0, 0, kv, "kvt")

        # ---- attention + out-projection pipeline over qc ----
        sp = ctx.enter_context(tc.tile_pool(name="sp", bufs=3, space="PSUM"))
        cp = ctx.enter_context(tc.tile_pool(name="cp", bufs=1, space="PSUM"))
        ep = ctx.enter_context(tc.tile_pool(name="ep", bufs=12))
        npool = ctx.enter_context(tc.tile_pool(name="npool", bufs=3))
        op = ctx.enter_context(tc.tile_pool(name="op", bufs=4))

        def outproj_si(qc, si, pool=None, pname="sblk", evict=None):
            ssl = slice(si * 128, (si + 1) * 128)
            os_ = op.tile([128, D], F16, name="os_")
            ps = (pool or sp).tile([128, 2, 512], F32, name=pname)
            for eh in range(2):
                for dc in range(2):
                    nc.tensor.matmul(
                        ps[:, eh, :],
                        ctx_q[qc][dc][:, ssl],
                        woT_s[:, dc, eh * 512:(eh + 1) * 512],
                        start=(dc == 0), stop=(dc == 1))
            if evict == "scalar":
                nc.scalar.copy(os_[:], ps[:].rearrange("p a b -> p (a b)"))
            else:
                nc.vector.tensor_copy(os_[:], ps[:].rearrange("p a b -> p (a b)"))
            row = qc * 512 + si * 128
            eng = nc.sync if (si % 2 == 0 or qc == QC - 1) else nc.gpsimd
            eng.dma_start(out_d[row:row + 128, :], os_[:])

        DELAY = 10
        NU = QC * 2 * KT        # 128 global attention units
        cps = {}
        exs = {}

        def do_scores(u):
            qc, pair, kt = u >> 5, (u >> 4) & 1, u & 15
            if kt == 0:
                cps[(qc, pair)] = cp.tile([128, 2, 512], F32, name="cps")
            sblk = sp.tile([128, 2, 512], F32, name="sblk")
            for hi in range(2):
                po = hi * 64
                nc.tensor.matmul(
                    sblk[:, hi, :],
                    KT_p[pair * 2 + kt // 8][po:po + 64,
                                             (kt % 8) * 128:(kt % 8 + 1) * 128],
                    QT_q[qc][po:po + 64, pair, :],
                    start=True, stop=True)
            ex = ep.tile([128, 2, 512], F16, name="ex")
            nc.scalar.activation(ex[:], sblk[:], EXP, scale=0.125)
            exs[u] = ex

        def do_ctx(u):
            qc, pair, kt = u >> 5, (u >> 4) & 1, u & 15
            ex = exs.pop(u)
            for hi in range(2):
                h = pair * 2 + hi
                nc.tensor.matmul(
                    cps[(qc, pair)][:, hi, :],
                    Vp_t[kt // 4][:, kt % 4, h * 128:(h + 1) * 128],
                    ex[:, hi, :],
                    start=(kt == 0), stop=(kt == KT - 1))
            if kt == KT - 1:
                finish_pair(qc, pair)

        def finish_pair(qc, pair):
            last = (qc == QC - 1 and pair == 1)
            cc = cps.pop((qc, pair))
            if last:
                cu = cc  # read psum directly; no next pair needs the slot
            else:
                cu = npool.tile([128, 2, 512], F32, name="cu")
                nc.vector.tensor_copy(cu[:], cc[:, :, :])
            rc = npool.tile([1, 1024], F32, name="rc")
            nc.vector.reciprocal_approx_fast(
                rc[:], cu[0:1, :, :].rearrange("p a b -> p (a b)"))
            bc = npool.tile([128, 1024], F32, name="bc")
            for hi in range(2):
                nc.gpsimd.partition_broadcast(
                    bc[:, hi * 512:(hi + 1) * 512],
                    rc[:, hi * 512:(hi + 1) * 512])
                po = hi * 64
                nc.vector.tensor_mul(
                    ctx_q[qc][pair][po:po + 64, :], cu[64:128, hi, :],
                    bc[64:128, hi * 512:(hi + 1) * 512])

        # Filler schedule (global stream position -> projection/outproj work).
        # Deadlines: KT_p[1] by u8, KT_p[2] by u16, KT_p[3] by u24,
        # vproj_sg(s) by ctx kt=2s (position 2s+DELAY), QT_q[q] by u=32q,
        # outproj(qc) after finish_pair(qc,1) (position 32qc+31+DELAY).
        def KPH(ec, kq, half):
            return lambda: kproj_half(ec, kq, half, sp, "sblk")

        def VPH(sg, half):
            return lambda: vproj_half(sg, half, sp, "sblk")

        fillers = {
            1: lambda: qproj_ec(0, 1),
            2: VPH(0, 0), 3: VPH(0, 1),
            4: VPH(1, 0), 5: VPH(1, 1),
            6: KPH(0, 1, 0), 7: KPH(0, 1, 1),
            8: VPH(2, 0), 9: VPH(2, 1),
            10: VPH(3, 0), 11: VPH(3, 1),
            12: KPH(1, 0, 0), 13: KPH(1, 0, 1),
            14: VPH(4, 0), 15: VPH(4, 1),
            16: VPH(5, 0), 17: VPH(5, 1),
            18: VPH(6, 0), 19: VPH(6, 1),
            20: VPH(7, 0), 21: VPH(7, 1),
            22: KPH(1, 1, 0), 23: KPH(1, 1, 1),
            25: lambda: qproj_ec(1, 0),
            27: lambda: qproj_ec(1, 1),
        }
        for qc in range(1, QC):
            base = 32 * qc + 11
            for si in range(4):
                fillers[base + 4 * si] = (
                    lambda q=qc - 1, s=si: outproj_si(q, s))
            if qc + 1 < QC:
                fillers[base + 14] = lambda q=qc + 1: qproj_ec(q, 0)
                fillers[base + 16] = lambda q=qc + 1: qproj_ec(q, 1)

        def outproj_begin(qc, si):
            ps = sp.tile([128, 2, 512], F32, name="sblk")
            for eh in range(2):
                nc.tensor.matmul(
                    ps[:, eh, :], ctx_q[qc][0][:, si * 128:(si + 1) * 128],
                    woT_s[:, 0, eh * 512:(eh + 1) * 512],
                    start=True, stop=False)
            return ps

        def outproj_end(qc, si, ps, evict=None):
            os_ = op.tile([128, D], F16, name="os_")
            for eh in range(2):
                nc.tensor.matmul(
                    ps[:, eh, :], ctx_q[qc][1][:, si * 128:(si + 1) * 128],
                    woT_s[:, 1, eh * 512:(eh + 1) * 512],
                    start=False, stop=True)
            if evict == "scalar":
                nc.scalar.copy(os_[:], ps[:].rearrange("p a b -> p (a b)"))
            else:
                nc.vector.tensor_copy(os_[:], ps[:].rearrange("p a b -> p (a b)"))
            nc.sync.dma_start(
                out_d[qc * 512 + si * 128:qc * 512 + (si + 1) * 128, :], os_[:])

        held = {}
        for u in range(NU + DELAY):
            if u >= DELAY:
                do_ctx(u - DELAY)
            if u < NU:
                do_scores(u)
            if u in fillers:
                fillers[u]()
            # pre-run the dc=0 half of the last chunk's out-projection in the
            # ctx-only tail (pair (3,0) is normalized by then; ring is idle)
            if NU <= u < NU + 3:
                held[u - NU] = outproj_begin(QC - 1, u - NU)
        # finish after the last pair's normalization; evict on the now-idle
        # scalar engine so the DVE norm chain isn't in the critical path
        for si in range(3):
            outproj_end(QC - 1, si, held.pop(si), evict="scalar")
        outproj_si(QC - 1, 3, pool=sp, pname="sblk", evict="scalar")

    nc.compile()
    return nc


_STATE = {}


def _get_nc():
    if "nc" not in _STATE:
        _STATE["nc"] = _build()
    return _STATE["nc"]


def kernel(x, wq, bq, wk, bk, wv, bv, wo, bo):
    x = np.asarray(x, dtype=np.float32)
    wq = np.asarray(wq, dtype=np.float32)
    bq = np.asarray(bq, dtype=np.float32)
    wk = np.asarray(wk, dtype=np.float32)
    bk = np.asarray(bk, dtype=np.float32)
    wv = np.asarray(wv, dtype=np.float32)
    bv = np.asarray(bv, dtype=np.float32)
    wo = np.asarray(wo, dtype=np.float32)
    bo = np.asarray(bo, dtype=np.float32)

    nc = _get_nc()

    def _pswiz(wT):  # [D_or_E, M] -> [128, D//128, M] partition-major
        dd, m = wT.shape
        return np.ascontiguousarray(
            wT.reshape(dd // 128, 128, m).transpose(1, 0, 2)).astype(np.float16)

    in_maps = []
    for c in range(N_CORES):
        b, g = divmod(c, 4)
        cols = slice(g * E, (g + 1) * E)
        xT = x[b].T  # [D, S]
        xsw = np.ascontiguousarray(
            xT.reshape(8, 128, 2, 1024).transpose(1, 2, 0, 3)).astype(np.float16)
        in_maps.append({
            "xT": xsw,                             # [128, 2, 8, 1024]
            "wqT": _pswiz(wq[cols, :].T),          # [128, 8, 256]
            "wkT": _pswiz(wk[cols, :].T),
            "wvT": _pswiz(wv[cols, :].T),
            "woT": _pswiz(wo[:, cols].T),          # [128, 2, 1024]
            "bq": np.ascontiguousarray(bq[cols]),
            "bk": np.ascontiguousarray(bk[cols]),
        })

    from concourse import bass_utils
    res = bass_utils.run_bass_kernel_spmd(
        nc, in_maps, core_ids=list(range(N_CORES)), trace=False)

    bias = (bo + wo @ bv).astype(np.float32)
    out = np.empty((2, S, D), dtype=np.float32)
    for b in range(2):
        acc = res.results[b * 4 + 0]["out"].astype(np.float64)
        for g in range(1, 4):
            acc += res.results[b * 4 + g]["out"]
        out[b] = (acc + bias).astype(np.float32)
    return out



# revision 28
# speedup vs baseline: 1.0278x; 1.0278x over previous
"""Multi-head self-attention (B=2, S=2048, D=1024, H=16) on 8 trn2 cores.

Sharding: data-parallel over B (2) x tensor-parallel over head groups (4 groups
of 4 heads).  Core c handles batch c//4, heads (c%4)*4..(c%4)*4+3.  Each core
computes its partial output projection (over its 256 of the 1024 contraction
columns); the host sums the 4 partials per batch and adds the bias terms.

Structure (per core):
  - K/V projections upfront (2-bank PSUM slots, fused evictions).
  - Per 512-query chunk: scores (K=64 matmuls into a [128,2heads,512]
    pair-block), one 1024-wide exp on ScalarE per block, ctx accumulation
    (exp @ [V | ones] gives softmax denominators for free), normalization
    via reciprocal + GpSimd partition-broadcast.
  - The previous chunk's output projection is interleaved into the current
    chunk's kt loop so the in-order PE queue always has independent work
    (keeps the HAM clock gate at 8/8).
"""

import sys

if "/opt/trn_rl_repo" not in sys.path:
    sys.path.insert(0, "/opt/trn_rl_repo")

from contextlib import ExitStack

import ml_dtypes
import numpy as np

import concourse.bass as bass
import concourse.mybir as mybir
import concourse.tile as tile
from concourse import bacc

F32 = mybir.dt.float32
F32R = mybir.dt.float32r
BF16 = mybir.dt.bfloat16
F16 = mybir.dt.float16
EXP = mybir.ActivationFunctionType.Exp

N_CORES = 8
S = 2048          # sequence length
D = 1024          # model dim
GH = 4            # heads per core
DK = 64           # head dim
E = GH * DK       # projection cols per core (256)
DT = D // 128     # contraction tiles over model dim (8)
KT = S // 128     # key chunks (16)
QC = S // 512     # query chunks (4)
DT_ORDER = [4, 5, 6, 7, 0, 1, 2, 3]   # x dt-chunk arrival order (gpsimd first)


def _build():
    nc = bacc.Bacc("TRN2", target_bir_lowering=False, debug=False,
                   enable_asserts=False, num_devices=N_CORES)

    # Inputs are pre-swizzled host-side to partition-major layouts so every
    # DMA reads large contiguous per-partition lines from HBM.
    xT_d = nc.dram_tensor("xT", [128, 2, DT, 1024], F16, kind="ExternalInput").ap()
    wqT_d = nc.dram_tensor("wqT", [128, DT, E], F16, kind="ExternalInput").ap()
    wkT_d = nc.dram_tensor("wkT", [128, DT, E], F16, kind="ExternalInput").ap()
    wvT_d = nc.dram_tensor("wvT", [128, DT, E], F16, kind="ExternalInput").ap()
    woT_d = nc.dram_tensor("woT", [128, 2, D], F16, kind="ExternalInput").ap()
    bq_d = nc.dram_tensor("bq", [E], F32, kind="ExternalInput").ap()
    bk_d = nc.dram_tensor("bk", [E], F32, kind="ExternalInput").ap()
    out_d = nc.dram_tensor("out", [S, D], F16, kind="ExternalOutput").ap()

    with tile.TileContext(nc) as tc, ExitStack() as ctx:
        const = ctx.enter_context(tc.tile_pool(name="const", bufs=1))

        xT_s = const.tile([128, DT, S], F16, name="xT_s")
        wqT_s = const.tile([128, DT, E], F16, name="wqT_s")
        wkT_s = const.tile([128, DT, E], F16, name="wkT_s")
        wvT_s = const.tile([128, DT, E], F16, name="wvT_s")
        woT_s = const.tile([128, 2, D], F16, name="woT_s")
        bq_s = const.tile([128, 2], F32, name="bq_s")
        bk_s = const.tile([128, 2], F32, name="bk_s")
        # Q and ctx live in per-qc tiles to avoid cross-qc WAR serialization
        QT_q = [const.tile([128, 2, 512], F16, name=f"QT{i}") for i in range(QC)]
        ctx_q = [[const.tile([128, 512], F16, name=f"CT{i}_{p}") for p in range(2)]
                 for i in range(QC)]
        KT_p = [const.tile([128, 1024], F16, name=f"KTp{i}") for i in range(4)]
        # V' layout: [k-partition, k-chunk, head-major (64 V cols + ones col)]
        Vp_t = [const.tile([128, 4, GH * 128], F16, name=f"Vp{i}")
                for i in range(4)]

        # Batched DMAs: contiguous >=512KB per-partition-major sources, split
        # across the two DMA queues so x-lo (kproj00/qproj0) lands first,
        # x-hi next (kproj01 at ~u7 of the stream), weights by first use.
        nc.sync.dma_start(wkT_s[:, 4:8, :], wkT_d[:, 4:8, :])
        nc.sync.dma_start(wkT_s[:, 0:4, :], wkT_d[:, 0:4, :])
        nc.sync.dma_start(bk_s[:, :], bk_d.rearrange("(c p) -> p c", p=128))
        nc.sync.dma_start(bq_s[:, :], bq_d.rearrange("(c p) -> p c", p=128))
        nc.sync.dma_start(xT_s[:, 0:2, 0:1024], xT_d[:, 0, 0:2, :])
        nc.sync.dma_start(xT_s[:, 2:4, 0:1024], xT_d[:, 0, 2:4, :])
        nc.sync.dma_start(xT_s[:, 0:2, 1024:2048], xT_d[:, 1, 0:2, :])
        nc.sync.dma_start(xT_s[:, 2:4, 1024:2048], xT_d[:, 1, 2:4, :])
        nc.gpsimd.dma_start(xT_s[:, 4, 0:1024], xT_d[:, 0, 4, :])
        nc.gpsimd.dma_start(xT_s[:, 5, 0:1024], xT_d[:, 0, 5, :])
        nc.gpsimd.dma_start(xT_s[:, 6:8, 0:1024], xT_d[:, 0, 6:8, :])
        nc.gpsimd.dma_start(wqT_s[:, 4:8, :], wqT_d[:, 4:8, :])
        nc.gpsimd.dma_start(wqT_s[:, 0:4, :], wqT_d[:, 0:4, :])
        nc.gpsimd.dma_start(wvT_s[:], wvT_d[:])
        nc.gpsimd.dma_start(xT_s[:, 4:6, 1024:2048], xT_d[:, 1, 4:6, :])
        nc.gpsimd.dma_start(xT_s[:, 6:8, 1024:2048], xT_d[:, 1, 6:8, :])
        nc.gpsimd.dma_start(woT_s[:], woT_d[:])

        # V' per head: [ones, 63 x zero, V(64)] -> denom at psum partition 0,
        # ctx at partitions 64..127 (32-aligned for DVE reads)
        for vt in Vp_t:
            nc.gpsimd.memset(vt[:], 0.0)
            for hh in range(GH):
                nc.vector.memset(vt[:, :, hh * 128], 1.0)

        def kproj_half(ec, kq, half, pool, pname):
            ps = pool.tile([128, 2, 512], F32, name=pname)
            qc = kq * 2 + half
            for i, dt_ in enumerate(DT_ORDER):
                nc.tensor.matmul(
                    ps[:, 0, :],
                    wkT_s[:, dt_, ec * 128:(ec + 1) * 128],
                    xT_s[:, dt_, qc * 512:(qc + 1) * 512],
                    start=(i == 0), stop=(i == DT - 1))
            nc.vector.tensor_scalar_add(
                KT_p[ec * 2 + kq].rearrange("p (a b) -> p a b", a=2)[:, half, :],
                ps[:, 0, :], bk_s[:, ec:ec + 1])

        def kproj_kq(ec, kq, pool, pname):
            kproj_half(ec, kq, 0, pool, pname)
            kproj_half(ec, kq, 1, pool, pname)

        def vproj_half(sg, half, pool, pname):
            ps = pool.tile([128, 2, 512], F32, name=pname)
            sc = sg * 2 + half
            for i, dt_ in enumerate(DT_ORDER):
                nc.tensor.matmul(
                    ps[:, 0, :E],
                    xT_s[:, dt_, sc * 128:(sc + 1) * 128],
                    wvT_s[:, dt_, :],
                    start=(i == 0), stop=(i == DT - 1))
            vt, so = Vp_t[sg // 2], (sg % 2) * 2 + half
            nc.vector.tensor_copy(
                vt[:, so:so + 1, :]
                .rearrange("p s (h d) -> p s h d", d=128)[:, :, :, DK:128],
                ps[:, 0:1, :E].rearrange("p s (h d) -> p s h d", d=DK))

        def vproj_sg(sg, pool, pname):
            vproj_half(sg, 0, pool, pname)
            vproj_half(sg, 1, pool, pname)

        def qproj_ec(qc, ec, pool=None, pname="sblk"):
            ps = (pool or sp).tile([128, 2, 512], F32, name=pname)
            for i, dt_ in enumerate(DT_ORDER):
                nc.tensor.matmul(
                    ps[:, ec, :],
                    wqT_s[:, dt_, ec * 128:(ec + 1) * 128],
                    xT_s[:, dt_, qc * 512:(qc + 1) * 512],
                    start=(i == 0), stop=(i == DT - 1))
            nc.vector.tensor_scalar_add(
                QT_q[qc][:, ec, :], ps[:, ec, :], bq_s[:, ec:ec + 1])

        def qproj(qc, pool=None, pname="sblk"):
            qproj_ec(qc, 0, pool, pname)
            qproj_ec(qc, 1, pool, pname)

        # ---- upfront projections (own 2-slot psum pool, closed afterwards) ----
        with tc.tile_pool(name="kv", bufs=2, space="PSUM") as kv:
            kproj_kq(0, 0, kv, "kvt")
            qproj_ec(0, 0, kv, "kvt")

        # ---- attention + out-projection pipeline over qc ----
        sp = ctx.enter_context(tc.tile_pool(name="sp", bufs=3, space="PSUM"))
        cp = ctx.enter_context(tc.tile_pool(name="cp", bufs=1, space="PSUM"))
        ep = ctx.enter_context(tc.tile_pool(name="ep", bufs=12))
        npool = ctx.enter_context(tc.tile_pool(name="npool", bufs=3))
        op = ctx.enter_context(tc.tile_pool(name="op", bufs=4))

        def outproj_si(qc, si, pool=None, pname="sblk", evict=None):
            ssl = slice(si * 128, (si + 1) * 128)
            os_ = op.tile([128, D], F16, name="os_")
            ps = (pool or sp).tile([128, 2, 512], F32, name=pname)
            for eh in range(2):
                for dc in range(2):
                    nc.tensor.matmul(
                        ps[:, eh, :],
                        ctx_q[qc][dc][:, ssl],
                        woT_s[:, dc, eh * 512:(eh + 1) * 512],
                        start=(dc == 0), stop=(dc == 1))
            if evict == "scalar":
                nc.scalar.copy(os_[:], ps[:].rearrange("p a b -> p (a b)"))
            else:
                nc.vector.tensor_copy(os_[:], ps[:].rearrange("p a b -> p (a b)"))
            row = qc * 512 + si * 128
            eng = nc.sync if (si % 2 == 0 or qc == QC - 1) else nc.gpsimd
            eng.dma_start(out_d[row:row + 128, :], os_[:])

        DELAY = 8
        NU = QC * 2 * KT        # 128 global attention units
        cps = {}
        exs = {}

        def do_scores(u):
            qc, pair, kt = u >> 5, (u >> 4) & 1, u & 15
            if kt == 0:
                cps[(qc, pair)] = cp.tile([128, 2, 512], F32, name="cps")
            sblk = sp.tile([128, 2, 512], F32, name="sblk")
            for hi in range(2):
                po = hi * 64
                nc.tensor.matmul(
                    sblk[:, hi, :],
                    KT_p[pair * 2 + kt // 8][po:po + 64,
                                             (kt % 8) * 128:(kt % 8 + 1) * 128],
                    QT_q[qc][po:po + 64, pair, :],
                    start=True, stop=True)
            ex = ep.tile([128, 2, 512], F16, name="ex")
            nc.scalar.activation(ex[:], sblk[:], EXP, scale=0.125)
            exs[u] = ex

        def do_ctx(u):
            qc, pair, kt = u >> 5, (u >> 4) & 1, u & 15
            ex = exs.pop(u)
            for hi in range(2):
                h = pair * 2 + hi
                nc.tensor.matmul(
                    cps[(qc, pair)][:, hi, :],
                    Vp_t[kt // 4][:, kt % 4, h * 128:(h + 1) * 128],
                    ex[:, hi, :],
                    start=(kt == 0), stop=(kt == KT - 1))
            if kt == KT - 1:
                finish_pair(qc, pair)

        def finish_pair(qc, pair):
            last = (qc == QC - 1 and pair == 1)
            cc = cps.pop((qc, pair))
            if last:
                cu = cc  # read psum directly; no next pair needs the slot
            else:
                cu = npool.tile([128, 2, 512], F32, name="cu")
                nc.vector.tensor_copy(cu[:], cc[:, :, :])
            rc = npool.tile([1, 1024], F32, name="rc")
            nc.vector.reciprocal_approx_fast(
                rc[:], cu[0:1, :, :].rearrange("p a b -> p (a b)"))
            bc = npool.tile([128, 1024], F32, name="bc")
            for hi in range(2):
                nc.gpsimd.partition_broadcast(
                    bc[:, hi * 512:(hi + 1) * 512],
                    rc[:, hi * 512:(hi + 1) * 512])
                po = hi * 64
                nc.vector.tensor_mul(
                    ctx_q[qc][pair][po:po + 64, :], cu[64:128, hi, :],
                    bc[64:128, hi * 512:(hi + 1) * 512])

        # Filler schedule (global stream position -> projection/outproj work).
        # Deadlines: KT_p[1] by u8, KT_p[2] by u16, KT_p[3] by u24,
        # vproj_sg(s) by ctx kt=2s (position 2s+DELAY), QT_q[q] by u=32q,
        # outproj(qc) after finish_pair(qc,1) (position 32qc+31+DELAY).
        def KPH(ec, kq, half):
            return lambda: kproj_half(ec, kq, half, sp, "sblk")

        def VPH(sg, half):
            return lambda: vproj_half(sg, half, sp, "sblk")

        fillers = {
            1: lambda: qproj_ec(0, 1),
            2: VPH(0, 0), 3: VPH(0, 1),
            4: VPH(1, 0), 5: VPH(1, 1),
            6: KPH(0, 1, 0), 7: KPH(0, 1, 1),
            8: VPH(2, 0), 9: VPH(2, 1),
            10: VPH(3, 0), 11: VPH(3, 1),
            12: KPH(1, 0, 0), 13: KPH(1, 0, 1),
            14: VPH(4, 0), 15: VPH(4, 1),
            16: VPH(5, 0), 17: VPH(5, 1),
            18: VPH(6, 0), 19: VPH(6, 1),
            20: VPH(7, 0), 21: VPH(7, 1),
            22: KPH(1, 1, 0), 23: KPH(1, 1, 1),
            25: lambda: qproj_ec(1, 0),
            27: lambda: qproj_ec(1, 1),
        }
        for qc in range(1, QC):
            base = 32 * qc + 11
            for si in range(4):
                fillers[base + 4 * si] = (
                    lambda q=qc - 1, s=si: outproj_si(q, s))
            if qc + 1 < QC:
                fillers[base + 14] = lambda q=qc + 1: qproj_ec(q, 0)
                fillers[base + 16] = lambda q=qc + 1: qproj_ec(q, 1)

        def outproj_begin(qc, si):
            ps = sp.tile([128, 2, 512], F32, name="sblk")
            for eh in range(2):
                nc.tensor.matmul(
                    ps[:, eh, :], ctx_q[qc][0][:, si * 128:(si + 1) * 128],
                    woT_s[:, 0, eh * 512:(eh + 1) * 512],
                    start=True, stop=False)
            return ps

        def outproj_end(qc, si, ps, evict=None):
            os_ = op.tile([128, D], F16, name="os_")
            for eh in range(2):
                nc.tensor.matmul(
                    ps[:, eh, :], ctx_q[qc][1][:, si * 128:(si + 1) * 128],
                    woT_s[:, 1, eh * 512:(eh + 1) * 512],
                    start=False, stop=True)
            if evict == "scalar":
                nc.scalar.copy(os_[:], ps[:].rearrange("p a b -> p (a b)"))
            else:
                nc.vector.tensor_copy(os_[:], ps[:].rearrange("p a b -> p (a b)"))
            nc.sync.dma_start(
                out_d[qc * 512 + si * 128:qc * 512 + (si + 1) * 128, :], os_[:])

        held = {}
        for u in range(NU + DELAY):
            if u >= DELAY:
                do_ctx(u - DELAY)
            if u < NU:
                do_scores(u)
            if u in fillers:
                fillers[u]()
            # pre-run the dc=0 half of the last chunk's out-projection in the
            # ctx-only tail (pair (3,0) is normalized by then; ring is idle)
            if NU <= u < NU + 3:
                held[u - NU] = outproj_begin(QC - 1, u - NU)
        # finish after the last pair's normalization; evict on the now-idle
        # scalar engine so the DVE norm chain isn't in the critical path
        for si in range(3):
            outproj_end(QC - 1, si, held.pop(si), evict="scalar")
        outproj_si(QC - 1, 3, pool=sp, pname="sblk", evict="scalar")

    nc.compile()
    return nc


_STATE = {}


def _get_nc():
    if "nc" not in _STATE:
        _STATE["nc"] = _build()
    return _STATE["nc"]


def kernel(x, wq, bq, wk, bk, wv, bv, wo, bo):
    x = np.asarray(x, dtype=np.float32)
    wq = np.asarray(wq, dtype=np.float32)
    bq = np.asarray(bq, dtype=np.float32)
    wk = np.asarray(wk, dtype=np.float32)
    bk = np.asarray(bk, dtype=np.float32)
    wv = np.asarray(wv, dtype=np.float32)
    bv = np.asarray(bv, dtype=np.float32)
    wo = np.asarray(wo, dtype=np.float32)
    bo = np.asarray(bo, dtype=np.float32)

    nc = _get_nc()

    def _pswiz(wT):  # [D_or_E, M] -> [128, D//128, M] partition-major
        dd, m = wT.shape
        return np.ascontiguousarray(
            wT.reshape(dd // 128, 128, m).transpose(1, 0, 2)).astype(np.float16)

    in_maps = []
    for c in range(N_CORES):
        b, g = divmod(c, 4)
        cols = slice(g * E, (g + 1) * E)
        xT = x[b].T  # [D, S]
        xsw = np.ascontiguousarray(
            xT.reshape(8, 128, 2, 1024).transpose(1, 2, 0, 3)).astype(np.float16)
        in_maps.append({
            "xT": xsw,                             # [128, 2, 8, 1024]
            "wqT": _pswiz(wq[cols, :].T),          # [128, 8, 256]
            "wkT": _pswiz(wk[cols, :].T),
            "wvT": _pswiz(wv[cols, :].T),
            "woT": _pswiz(wo[:, cols].T),          # [128, 2, 1024]
            "bq": np.ascontiguousarray(bq[cols]),
            "bk": np.ascontiguousarray(bk[cols]),
        })

    from concourse import bass_utils
    res = bass_utils.run_bass_kernel_spmd(
        nc, in_maps, core_ids=list(range(N_CORES)), trace=False)

    bias = (bo + wo @ bv).astype(np.float32)
    out = np.empty((2, S, D), dtype=np.float32)
    for b in range(2):
        acc = res.results[b * 4 + 0]["out"].astype(np.float64)
        for g in range(1, 4):
            acc += res.results[b * 4 + g]["out"]
        out[b] = (acc + bias).astype(np.float32)
    return out

